# revision 32
# baseline (speedup 1.0000x reference)
"""GCN block (self-loop + sym-norm + linear + scatter-add + bias + relu) on 8 trn2 cores.

Sharding: nodes partitioned across cores by destination range. Each core
gathers x[src] rows for its incoming edges directly from its HBM copy of x
via dma_gather, aggregates per 128-node destination tile with one-hot
matmuls on the tensor engine, then applies the symmetric normalization and
the linear layer.

Host does integer-only graph preprocessing (CSR-style bucketing, degree
counts, padding, int16 index wrapping); all floating-point math runs on
device.
"""

import math
import os
import sys

import numpy as np

sys.path.insert(0, "/opt/trn_rl_repo")

import concourse.bacc as bacc
import concourse.bass as bass
import concourse.mybir as mybir
import concourse.tile as tile
from concourse.bass_utils import run_bass_kernel_spmd

F32 = mybir.dt.float32
BF16 = mybir.dt.bfloat16
I16 = mybir.dt.int16

N_CORES = 8
P = 128            # node-tile width / partition count
SUPER = 4          # node tiles per gather super-tile
LO_LIM = 32768     # int16 positive range for gather indices
SINGLE_PACKET = False


# ----------------------------------------------------------------------------
# host-side integer preprocessing
# ----------------------------------------------------------------------------

def _wrap_idxs(idx):
    """[n] int16 (n % 128 == 0) -> [128, n//16] wrapped+replicated layout."""
    n = idx.shape[0]
    arr = idx.reshape(n // 16, 16).T  # [16, cols]; arr[p, s] = idx[s*16+p]
    return np.tile(arr, (8, 1))


def preprocess(edge_index, n_nodes):
    """Bucket edges (incl. self-loops) by (core, dest tile); pad to 128-edge
    blocks split into lo/hi src halves. Returns per-core arrays + static meta.
    """
    E = edge_index.shape[1]
    src = np.concatenate([edge_index[0], np.arange(n_nodes, dtype=np.int64)])
    dst = np.concatenate([edge_index[1], np.arange(n_nodes, dtype=np.int64)])

    deg = np.bincount(dst, minlength=n_nodes).astype(np.int64)  # >= 1 (loops)

    npc = (n_nodes + N_CORES - 1) // N_CORES          # nodes per core
    T = (npc + P - 1) // P                            # tiles per core
    core = dst // npc
    d_local = dst - core * npc
    t_of = d_local // P
    dst_local = d_local % P
    is_lo = src < LO_LIM

    # sort edges by (core, tile, lo/hi) once; stable order inside groups
    order = np.lexsort((~is_lo, t_of, core))
    src_s, core_s, t_s, dl_s, lo_s = (
        src[order], core[order], t_of[order], dst_local[order], is_lo[order])

    # per (core, tile) lo/hi counts -> per-tile block counts (max over cores)
    key = (core_s * T + t_s) * 2 + (~lo_s).astype(np.int64)
    cnt = np.bincount(key, minlength=N_CORES * T * 2).reshape(N_CORES, T, 2)
    nb = np.ceil(cnt / P).astype(np.int64)            # blocks per (c, t, lo/hi)
    nb_lo = nb[:, :, 0].max(axis=0)                   # [T] uniform across cores
    nb_hi = nb[:, :, 1].max(axis=0)                   # [T]

    # super-tile grouping
    supers = [list(range(s, min(s + SUPER, T))) for s in range(0, T, SUPER)]

    # global block layout: per super-tile: [lo blocks by tile][hi blocks by tile]
    # record, per tile: (lo_block_start, nb_lo, hi_block_start, nb_hi) global idx
    blk_of_tile = {}
    calls = []      # per super-tile: (lo_nidx, hi_nidx, blk_start, nb_total)
    B = 0
    for S in supers:
        b0 = B
        lo_starts = {}
        for t in S:
            lo_starts[t] = B
            B += int(nb_lo[t])
        n_lo_blocks = B - b0
        for t in S:
            blk_of_tile[t] = (lo_starts[t], int(nb_lo[t]), B, int(nb_hi[t]))
            B += int(nb_hi[t])
        calls.append((n_lo_blocks * P, (B - b0 - n_lo_blocks) * P, b0, B - b0))
    NB = B  # total blocks per core

    # per-core padded slot arrays
    idx_cols = sum((lo + hi) // 16 for lo, hi, _, _ in calls)
    idx_all = np.zeros((N_CORES, P, idx_cols), np.int16)
    dl_all = np.full((N_CORES, P, NB), -1.0, np.float32)   # cast to bf16 later
    degsrc_all = np.ones((N_CORES, P, NB), np.float32)

    # group boundaries in the sorted edge array
    grp_start = np.zeros(N_CORES * T * 2 + 1, np.int64)
    np.cumsum(np.bincount(key, minlength=N_CORES * T * 2), out=grp_start[1:])

    for c in range(N_CORES):
        col = 0
        for (S, (lo_nidx, hi_nidx, b0, nbS)) in zip(supers, calls):
            for half, nidx in ((0, lo_nidx), (1, hi_nidx)):
                half_start = b0 if half == 0 else b0 + lo_nidx // P
                flat_idx = np.zeros(nidx, np.int64)
                pos = 0
                for t in S:
                    g = (c * T + t) * 2 + half
                    s0, s1 = grp_start[g], grp_start[g + 1]
                    cnt_g = s1 - s0
                    nb_g = int(nb_lo[t] if half == 0 else nb_hi[t])
                    sl = slice(pos, pos + cnt_g)
                    flat_idx[sl] = src_s[s0:s1] - (0 if half == 0 else LO_LIM)
                    # block-slot arrays: slot j -> (partition j%P, block j//P)
                    j = np.arange(pos, pos + cnt_g)
                    bcol = half_start + j // P
                    prow = j % P
                    dl_all[c, prow, bcol] = dl_s[s0:s1]
                    degsrc_all[c, prow, bcol] = deg[src_s[s0:s1]]
                    pos += nb_g * P
                if nidx:
                    idx_all[c, :, col:col + nidx // 16] = _wrap_idxs(
                        flat_idx.astype(np.int16))
                    col += nidx // 16
        assert col == idx_cols

    # per-core degree of own nodes, tile layout [P, T]
    deg_tile = np.ones((N_CORES, P, T), np.float32)
    for c in range(N_CORES):
        n0 = c * npc
        n1 = min(n0 + npc, n_nodes)
        own = deg[n0:n1].astype(np.float32)
        dt = np.ones(T * P, np.float32)
        dt[:own.shape[0]] = own
        deg_tile[c] = dt.reshape(T, P).T

    nbs_max = max(nbS for _, _, _, nbS in calls)
    meta = dict(npc=npc, T=T, NB=NB, idx_cols=idx_cols, supers=supers,
                calls=calls, blk_of_tile=blk_of_tile,
                nb_lo=nb_lo, nb_hi=nb_hi, nbs_max=nbs_max)
    arrays = dict(idx=idx_all, dst_local=dl_all, deg_src=degsrc_all,
                  deg_tile=deg_tile)
    return meta, arrays


# ----------------------------------------------------------------------------
# device program
# ----------------------------------------------------------------------------

def build_nc(n_nodes, d_in, d_out, meta, reps=1, mode="full", max_idx=896,
             n_queues=4, scratch=16384):
    T, NB, idx_cols = meta["T"], meta["NB"], meta["idx_cols"]
    NBS_MAX = meta["nbs_max"]
    out_rows = T * P

    nc = bacc.Bacc("TRN2", target_bir_lowering=False, debug=False,
                   num_swdge_queues=n_queues, dynamic_dma_scratch_size=scratch)

    x_d = nc.dram_tensor("x", [n_nodes, d_in], F32, kind="ExternalInput")
    waug_d = nc.dram_tensor("w_aug", [d_in + 1, d_out], F32, kind="ExternalInput")
    # c-major iota: iota_cb[p, c*NBS_MAX + j] = c. Unit-stride last dims keep
    # the one-hot compare/scale in the DVE 2x perf mode.
    iota_d = nc.dram_tensor("iota_cb", [P, P * NBS_MAX], BF16,
                            kind="ExternalInput")
    id01_d = nc.dram_tensor("id01", [P, P], F32, kind="ExternalInput")
    degt_d = nc.dram_tensor("deg_tile", [P, T], F32, kind="ExternalInput")
    degs_d = nc.dram_tensor("deg_src", [P, NB], F32, kind="ExternalInput")
    dl_d = nc.dram_tensor("dst_local", [P, NB], BF16, kind="ExternalInput")
    idx_d = nc.dram_tensor("idx", [P, idx_cols], I16, kind="ExternalInput")
    out_d = nc.dram_tensor("out", [out_rows, d_out], F32, kind="ExternalOutput")

    with tile.TileContext(nc) as tc:
        with (
            tc.tile_pool(name="const", bufs=1) as cpool,
            tc.tile_pool(name="gather", bufs=2) as gpool,
            tc.tile_pool(name="pmat", bufs=2) as ppool,
            tc.tile_pool(name="small", bufs=3) as spool,
            tc.tile_pool(name="psum", bufs=2, space="PSUM") as psum,
        ):
            # ---- constants / one-shot prep ----
            iota_sb = cpool.tile([P, P * NBS_MAX], BF16, tag="iota")
            nc.sync.dma_start(iota_sb[:, :], iota_d[:, :])
            iota3 = iota_sb[:, :].rearrange("p (c j) -> p c j", j=NBS_MAX)
            id01_sb = cpool.tile([P, P], F32, tag="id01")
            nc.sync.dma_start(id01_sb[:, :], id01_d[:, :])
            waug_sb = cpool.tile([d_in + 1, d_out], F32, tag="waug")
            nc.sync.dma_start(waug_sb[:, :], waug_d[:, :])
            dl_sb = cpool.tile([P, NB], BF16, tag="dl")
            nc.sync.dma_start(dl_sb[:, :], dl_d[:, :])
            idx_sb = cpool.tile([P, idx_cols], I16, tag="idx")
            nc.sync.dma_start(idx_sb[:, :], idx_d[:, :])

            def rsqrt(pool, deg_dram, cols, tag):
                """fp32 1/sqrt(deg): ACT sqrt + DVE reciprocal + 1 Newton."""
                d = pool.tile([P, cols], F32, tag=f"{tag}_d")
                nc.sync.dma_start(d[:, :], deg_dram[:, :])
                r = pool.tile([P, cols], F32, tag=f"{tag}_r")
                t1 = pool.tile([P, cols], F32, tag=f"{tag}_t")
                nc.scalar.sqrt(t1[:, :], d[:, :])
                nc.vector.reciprocal(r[:, :], t1[:, :])
                # newton: r <- r * (1.5 - 0.5 * d * r * r)
                nc.vector.tensor_mul(t1[:, :], r[:, :], r[:, :])
                nc.vector.tensor_mul(t1[:, :], t1[:, :], d[:, :])
                nc.vector.tensor_scalar(
                    out=t1[:, :], in0=t1[:, :], scalar1=-0.5, scalar2=1.5,
                    op0=mybir.AluOpType.mult, op1=mybir.AluOpType.add)
                nc.vector.tensor_mul(r[:, :], r[:, :], t1[:, :])
                return r

            dis_dst = rsqrt(cpool, degt_d, T, "degt")           # [P, T] fp32
            dis_src_f = rsqrt(cpool, degs_d, NB, "degs")        # [P, NB] fp32
            dis_src = cpool.tile([P, NB], BF16, tag="dis_src_bf")
            nc.vector.tensor_copy(dis_src[:, :], dis_src_f[:, :])

            ones_row = cpool.tile([1, P], F32, tag="ones")
            nc.vector.memset(ones_row[:, :], 1.0)

            out_sb = cpool.tile([P, T * d_out], F32, tag="out_sb")
            nc.vector.memset(out_sb[:, :], 0.0)

            # ---- main loop over gather super-tiles ----
            qstate = [0]

            def body():
              col = 0
              for S, (lo_nidx, hi_nidx, b0, nbS) in zip(meta["supers"], meta["calls"]):
                xg = gpool.tile([P, nbS * d_in], F32, tag="xg")
                xg3 = xg[:, :].rearrange("p (b e) -> p b e", e=d_in)
                if mode == "compute":
                    nc.gpsimd.memset(xg[:, :], 0.25)
                # SWDGE descriptor ring holds ~1024 descs per queue; one
                # dma_gather must fit, so chop each half into <=max_idx calls.
                halves = [(lo_nidx, x_d[:min(LO_LIM, n_nodes), :], 0)]
                if hi_nidx:
                    halves.append((hi_nidx, x_d[LO_LIM:, :], lo_nidx // P))
                for half_nidx, src_ap, blk0 in halves:
                    done = 0
                    while done < half_nidx:
                        n_i = min(max_idx, half_nidx - done)
                        b_lo = blk0 + done // P
                        if mode != "compute":
                            qstate[0] = (qstate[0] + 1) % n_queues
                            nc.gpsimd.dma_gather(
                                out_ap=xg3[:, b_lo:b_lo + n_i // P, :],
                                in_ap=src_ap,
                                idxs_ap=idx_sb[:, col:col + n_i // 16],
                                num_idxs=n_i, num_idxs_reg=n_i, elem_size=d_in,
                                queue_num=qstate[0], single_packet=SINGLE_PACKET)
                        col += n_i // 16
                        done += n_i
                if mode == "gather":
                    continue

                # cast gathered rows to bf16 (ACT)
                xgb = gpool.tile([P, nbS * d_in], BF16, tag="xgb")
                nc.scalar.activation(
                    xgb[:, :], xg[:, :], mybir.ActivationFunctionType.Copy)

                # one-hot scatter matrices, c-major [p, c, b], scaled by
                # dis[src]. All operands have unit-stride last dim -> DVE 2x.
                dl3 = dl_sb[:, b0:b0 + nbS].rearrange(
                    "p (o b) -> p o b", o=1).to_broadcast([P, P, nbS])
                ds3 = dis_src[:, b0:b0 + nbS].rearrange(
                    "p (o b) -> p o b", o=1).to_broadcast([P, P, nbS])
                pw = ppool.tile([P, P * nbS], BF16, tag="pw")
                pw3 = pw[:, :].rearrange("p (c b) -> p c b", b=nbS)
                nc.vector.tensor_tensor(
                    out=pw3, in0=dl3, in1=iota3[:, :, :nbS],
                    op=mybir.AluOpType.is_equal)
                nc.vector.tensor_tensor(
                    out=pw3, in0=pw3, in1=ds3, op=mybir.AluOpType.mult)

                for t in S:
                    lo_b, n_lo, hi_b, n_hi = meta["blk_of_tile"][t]
                    blocks = list(range(lo_b, lo_b + n_lo)) + \
                             list(range(hi_b, hi_b + n_hi))
                    ps = psum.tile([P, d_out], F32, tag="ps")
                    for i, b in enumerate(blocks):
                        rb = b - b0
                        nc.tensor.matmul(
                            ps[:, :],
                            lhsT=pw3[:, :, rb],
                            rhs=xgb[:, rb * d_in:(rb + 1) * d_in],
                            start=(i == 0), stop=(i == len(blocks) - 1))

                    # s -> sbuf scaled by dis[dst] (per-partition ACT scale),
                    # then transpose on PE with a true identity
                    s_sb = spool.tile([P, d_out], F32, tag="s_sb")
                    nc.scalar.activation(
                        s_sb[:, :], ps[:, :], mybir.ActivationFunctionType.Copy,
                        scale=dis_dst[:, t:t + 1])
                    pst = psum.tile([d_out, P], F32, tag="pst")
                    nc.tensor.transpose(pst[:, :], s_sb[:, :], id01_sb[:, :])

                    sT = spool.tile([d_in + 1, P], F32, tag="sT")
                    nc.scalar.activation(
                        sT[:d_out, :], pst[:, :], mybir.ActivationFunctionType.Copy)
                    nc.vector.tensor_copy(sT[d_in:d_in + 1, :], ones_row[:, :])

                    po = psum.tile([P, d_out], F32, tag="po")
                    nc.tensor.matmul(po[:, :], lhsT=sT[:, :], rhs=waug_sb[:, :],
                                     start=True, stop=True)
                    nc.scalar.activation(
                        out_sb[:, t * d_out:(t + 1) * d_out], po[:, :],
                        mybir.ActivationFunctionType.Relu)

            if reps == 1:
                body()
            else:
                with tc.For_i(0, reps, 1):
                    body()

            nc.sync.dma_start(
                out_d[:, :].rearrange("(t p) f -> p t f", p=P),
                out_sb[:, :].rearrange("p (t f) -> p t f", f=d_out))

    nc.compile()
    return nc


# ----------------------------------------------------------------------------
# public entry point
# ----------------------------------------------------------------------------

_CACHE = {}


def _get_compiled(n_nodes, d_in, d_out, edge_index):
    key = (n_nodes, d_in, d_out,
           hash(edge_index.tobytes()) if edge_index.size < (1 << 24)
           else hash(edge_index[:, ::97].tobytes()))
    hit = _CACHE.get(key)
    if hit is None:
        meta, arrays = preprocess(np.asarray(edge_index, dtype=np.int64), n_nodes)
        nc = build_nc(n_nodes, d_in, d_out, meta)
        hit = (nc, meta, arrays)
        _CACHE[key] = hit
    return hit


def _make_in_maps(x, W, b, meta, arrays, d_in, d_out):
    w_aug = np.concatenate([np.asarray(W, np.float32),
                            np.asarray(b, np.float32)[None, :]], axis=0)
    import ml_dtypes
    nbs_max = meta["nbs_max"]
    iota_cb = np.repeat(np.arange(P, dtype=np.float32), nbs_max)  # [P*nbs_max]
    iota_cb = np.tile(iota_cb, (P, 1)).astype(ml_dtypes.bfloat16)
    id01 = np.eye(P, dtype=np.float32)
    in_maps = []
    for c in range(N_CORES):
        in_maps.append({
            "x": np.asarray(x, np.float32),
            "w_aug": w_aug,
            "iota_cb": iota_cb,
            "id01": id01,
            "deg_tile": arrays["deg_tile"][c],
            "deg_src": arrays["deg_src"][c],
            "dst_local": arrays["dst_local"][c].astype(ml_dtypes.bfloat16),
            "idx": arrays["idx"][c],
        })
    return in_maps


def run(x, edge_index, W, b, trace=False):
    n_nodes, d_in = x.shape
    d_out = W.shape[1]
    nc, meta, arrays = _get_compiled(n_nodes, d_in, d_out,
                                     np.asarray(edge_index, dtype=np.int64))
    in_maps = _make_in_maps(x, W, b, meta, arrays, d_in, d_out)
    res = run_bass_kernel_spmd(nc, in_maps, core_ids=list(range(N_CORES)),
                               trace=trace)
    npc = meta["npc"]
    parts = [res.results[c]["out"][:min(npc, n_nodes - c * npc)]
             for c in range(N_CORES)]
    out = np.concatenate(parts, axis=0).astype(np.float32)
    return out, res


def kernel(x, edge_index, W, b):
    out, _ = run(np.asarray(x), np.asarray(edge_index), np.asarray(W),
                 np.asarray(b))
    return out
